# revision 15
# baseline (speedup 1.0000x reference)
"""BERT self-attention (B=8, S=1024, D=1024, H=16, DH=64) on 8 Trainium2 cores.

Strategy: pure data-parallel over batch - each of the 8 cores runs the full
self-attention for one batch element. No collectives.

v4 layout (HW-measured evolution; fp8 datapaths were tried and rejected -
fp8 probs or fp8 projections push absmax rel err past the 2e-2 gate):
  - fp16 datapath on the PE (same streaming/load speed as bf16, ~8x less
    quantization error; all values are far below fp16 range limits).
  - context computed TRANSPOSED (ctx^T = V'^T @ P^T): the stationary tensor
    is the [128,65] V' block (65-row weight load) instead of the [128,128]
    P^T block per (row-block, k-block) - on HW stationary loads cost ~2
    cycles/row so natural-orientation context was ~44% of the attention
    phase.  ctx^T accumulates in PSUM [65,1024], is copied to SBUF fp16 and
    transposed back by the PE in [128,130] pair tiles (the trailing ones
    column of V' makes row 64 the softmax denominator, which the transpose
    delivers per-partition for the reciprocal+scale normalize).
  - 3-head software pipeline: head h runs scores+exp(h) | ctx^T matmuls
    (h-1) | transpose-back+normalize+DMA (h-2).  ACT exp (~1.13us per
    [128,1024] tile) paces; PE per-step work exceeds it so the PE never
    idles (keeping its 2.4GHz p-state - idle gaps drop it to 1.2GHz).
  - Q/K projections as dense blocks between heads, it-major with both
    512-col halves consecutive per weight (the second matmul skips the
    serial weight reload); Q/K biases folded into the PSUM->SBUF copies as
    per-partition tensor_scalar adds.
  - scores computed TRANSPOSED: S^T[k,q] so the attention mask is a
    per-partition bias folded with the 1/sqrt(DH) scale into the Exp.
  - input DMAs spread across the sync and ACT HWDGE queues; W fp32->fp16
    converts round-robin ACT/DVE/Pool; X fp32->fp16 on Pool feeding fp16 PE
    transposes.
  - PSUM: one mixed ring-3 pool of 2-bank slots (scores tiles, proj/V
    accumulators, transpose pairs, phase-1 X^T transposes) + a dedicated
    ring-1 [65,1024] ctx^T accumulator -> exactly 8 banks.

Built on bacc.Bacc: its compile() legalizes sync waits (1 wait/instruction
hardware limit) via move_matmul_waits_to_ldweights + generate_event_semaphores.
"""

import numpy as np

import concourse.bass as bass
import concourse.bacc as bacc
import concourse.mybir as mybir
import concourse.tile as tile
from concourse.bass_utils import run_bass_kernel_spmd
from concourse.masks import make_identity

F32 = mybir.dt.float32
FP16 = mybir.dt.float16

B, S, D, H = 8, 1024, 1024, 16
DH = D // H  # 64
P = 128
NT = S // P  # 8 tiles along any 1024 dim
SC = S // 512  # 2 chunks of 512
SCALE = 1.0 / float(np.sqrt(DH))
N_CORES = 8
VW = DH + 1  # 65: V block width per head (64 cols + ones col)
HG = 4  # heads per output-DMA group

PHASES = 7  # bitmask: 1=x^T, 2=+V proj, 4=+attention loop (profiling aid)


def emit_body(nc, dram, pools):
    (x_d, m_d, wq_d, bq_d, wk_d, bk_d, wv_d, bv_d, o_d) = dram
    (cst, xT_pool, qkT_pool, v_pool, wf_pool, wb_pool, p_pool, small_pool,
     og_pool, cT_pool, ps_big, ps_cT, ident) = pools

    # ---- per-body constants (mask / bias) ----
    mask_cols = cst.tile([P, NT], F32, name="mask_cols", tag="mask_cols")
    nc.sync.dma_start(out=mask_cols, in_=m_d.ap().rearrange("(g p) -> p g", p=P))
    ones_f32 = cst.tile([1, 512], F32, name="ones_f32", tag="ones_f32")
    nc.vector.memset(ones_f32, 1.0)
    ones_row = cst.tile([1, 512], FP16, name="ones_row", tag="ones_row")
    nc.vector.tensor_copy(ones_row, ones_f32)
    # bq/bk as [128, NT] per-partition columns (added in the proj copies)
    b_cols = {}
    for nm, hd in (("bq", bq_d), ("bk", bk_d)):
        t = cst.tile([P, NT], F32, name=f"bcol_{nm}", tag=f"bcol_{nm}")
        nc.sync.dma_start(out=t, in_=hd.ap().rearrange("(g p) -> p g", p=P))
        b_cols[nm] = t
    # bv as a [1, D] fp16 row (rank-1 matmul in the V projection)
    bvf = cst.tile([1, D], F32, name="bvf", tag="bvf")
    nc.sync.dma_start(out=bvf, in_=bv_d.ap().unsqueeze(0))
    bv_row = cst.tile([1, D], FP16, name="bv_row", tag="bv_row")
    nc.vector.tensor_copy(bv_row, bvf)

    if not PHASES & 1:
        return

    # ---- phase 1: X^T via fp16 PE transposes; W DMAs spread over queues,
    # fp32->fp16 converts round-robin ACT/DVE/Pool ----
    xT = []
    for it in range(NT):
        xT.append(xT_pool.tile([P, S], FP16, name=f"xT{it}", tag=f"xT{it}"))

    w_bf = {}
    w_src = (("wv", wv_d, nc.sync), ("wq", wq_d, nc.scalar),
             ("wk", wk_d, nc.scalar))
    for nm, _, _ in w_src:
        w_bf[nm] = [
            wb_pool.tile([P, D], FP16, name=f"{nm}b{it}", tag=f"{nm}b{it}")
            for it in range(NT)
        ]

    for st in range(NT):
        x_t = wf_pool.tile([P, D], F32, name="x_tile", tag="wf")
        nc.sync.dma_start(out=x_t, in_=x_d.ap()[st * P : (st + 1) * P, :])
        x_bf = wf_pool.tile([P, D], FP16, name="x_bf", tag="xbf")
        nc.gpsimd.tensor_copy(x_bf, x_t)
        for ih in range(NT // 2):
            pt = ps_big.tile([P, 2 * P], FP16, name="pt", tag="big")
            for j in range(2):
                it = 2 * ih + j
                nc.tensor.transpose(
                    pt[:, j * P : (j + 1) * P],
                    x_bf[:, it * P : (it + 1) * P],
                    ident,
                )
            dst0 = xT[2 * ih][:, st * P : (st + 1) * P]
            dst1 = xT[2 * ih + 1][:, st * P : (st + 1) * P]
            if (st + ih) % 2 == 0:
                nc.vector.tensor_copy(dst0, pt[:, 0:P])
                nc.vector.tensor_copy(dst1, pt[:, P : 2 * P])
            else:
                nc.scalar.copy(dst0, pt[:, 0:P])
                nc.scalar.copy(dst1, pt[:, P : 2 * P])

    ci = 0
    for nm, w_d, eng in w_src:
        for it in range(NT):
            wf = wf_pool.tile([P, D], F32, name=f"{nm}f", tag="wf")
            eng.dma_start(out=wf, in_=w_d.ap()[it * P : (it + 1) * P, :])
            conv = (nc.scalar.copy, nc.vector.tensor_copy,
                    nc.gpsimd.tensor_copy)[ci % 3]
            conv(w_bf[nm][it], wf)
            ci += 1

    if not PHASES & 2:
        fin = small_pool.tile([P, DH], F32, name="fin1", tag="bounce")
        nc.vector.tensor_copy(fin, xT[0][:, 0:DH])
        nc.sync.dma_start(out=o_d.ap()[0:P, 0:DH], in_=fin)
        return

    # ---- phase 2: V projection (natural orientation, fp16), trailing
    # ones column per 65-wide head block makes probs@V' emit the softmax
    # denominator for free ----
    v_sb = []
    for st in range(NT):
        v = v_pool.tile([P, H * VW], FP16, name=f"v{st}", tag=f"v{st}")
        nc.gpsimd.memset(v, 1.0)
        v_sb.append(v)
    for st in range(NT):
        mm = ps_big.tile([P, S], F32, name="mmv", tag="big")
        for it in range(NT):
            for jc in range(SC):
                nc.tensor.matmul(
                    mm[:, jc * 512 : (jc + 1) * 512],
                    lhsT=xT[it][:, st * P : (st + 1) * P],
                    rhs=w_bf["wv"][it][:, jc * 512 : (jc + 1) * 512],
                    start=(it == 0),
                    stop=False,
                )
        for jc in range(SC):
            nc.tensor.matmul(
                mm[:, jc * 512 : (jc + 1) * 512],
                lhsT=ones_row[0:1, 0:P],
                rhs=bv_row[0:1, jc * 512 : (jc + 1) * 512],
                start=False,
                stop=True,
            )
        dst = v_sb[st].rearrange("p (g c) -> p g c", c=VW)[:, :, 0:DH]
        src = mm.rearrange("p (g c) -> p g c", c=DH)
        nc.vector.tensor_copy(dst, src)

    if not PHASES & 4:
        fin = small_pool.tile([P, DH], F32, name="fin2", tag="bounce")
        nc.vector.tensor_copy(fin, v_sb[0][:, 0:DH])
        nc.sync.dma_start(out=o_d.ap()[0:P, 0:DH], in_=fin)
        return

    # ---- phase 3: attention, 3-head pipeline ----
    staging = {}

    def emit_proj(jt):
        """Dense Q^T/K^T projection for column block jt, fp16 out with the
        bias folded into the PSUM->SBUF copy."""
        outs = []
        for nm, bnm in (("wq", "bq"), ("wk", "bk")):
            dst = qkT_pool.tile([P, S], FP16, name=f"{nm}T{jt}", tag=f"{nm}T")
            mm = ps_big.tile([P, S], F32, name="mm", tag="big")
            for it in range(NT):
                for sc in range(SC):
                    nc.tensor.matmul(
                        mm[:, sc * 512 : (sc + 1) * 512],
                        lhsT=w_bf[nm][it][:, jt * P : (jt + 1) * P],
                        rhs=xT[it][:, sc * 512 : (sc + 1) * 512],
                        start=(it == 0),
                        stop=(it == NT - 1),
                    )
            nc.vector.tensor_scalar_add(dst, mm, b_cols[bnm][:, jt : jt + 1])
            outs.append(dst)
        return outs

    def emit_scores_exp_step(h, kt, qTj, kTj):
        ro = (h % 2) * DH
        sps = ps_big.tile([P, S], F32, name="sps", tag="big")
        for qc in range(SC):
            nc.tensor.matmul(
                sps[:, qc * 512 : (qc + 1) * 512],
                lhsT=kTj[ro : ro + DH, kt * P : (kt + 1) * P],
                rhs=qTj[ro : ro + DH, qc * 512 : (qc + 1) * 512],
                start=True,
                stop=True,
            )
        pt = p_pool.tile([P, S], FP16, name="pT", tag="pT")
        nc.scalar.activation(
            pt,
            sps,
            mybir.ActivationFunctionType.Exp,
            bias=mask_cols[:, kt : kt + 1],
            scale=SCALE,
        )
        return pt

    # pipeline state
    ctxT_ps = [None]  # PSUM [65, S] accumulator of the in-flight ctx^T
    tp_pair = [None]

    def ctx_mm_pairs(h1, pT):
        """Chunks: ctx^T(h1) += V'[kt]^T @ P^T[kt], one kt (both 512-col
        halves, shared stationary V' block) per chunk."""
        for kt in range(NT):
            def mk(kt=kt):
                def go():
                    if kt == 0:
                        ctxT_ps[0] = ps_cT.tile(
                            [VW, S], F32, name="ctxT", tag="ctxT"
                        )
                    for qc in range(SC):
                        nc.tensor.matmul(
                            ctxT_ps[0][:, qc * 512 : (qc + 1) * 512],
                            lhsT=v_sb[kt][:, h1 * VW : (h1 + 1) * VW],
                            rhs=pT[kt][:, qc * 512 : (qc + 1) * 512],
                            start=(kt == 0),
                            stop=(kt == NT - 1),
                        )
                return go
            yield mk()

    def emit_ctxT_copy():
        """ctx^T PSUM -> SBUF fp16 (frees the ring-1 accumulator)."""
        sb = cT_pool.tile([VW, S], FP16, name="cTsb", tag="cTsb")
        nc.vector.tensor_copy(sb, ctxT_ps[0])
        return sb

    def emit_out_qt(h2, cT_sb, qt):
        """Transpose-back one row block of ctx^T(h2) and normalize into the
        output staging tile; DMA per 4-head group."""
        g = h2 // HG
        if h2 % HG == 0 and qt == 0:
            for q2 in range(NT):
                staging[q2] = og_pool.tile(
                    [P, HG * DH], F32, name=f"og{q2}", tag=f"og{q2}"
                )
        if qt % 2 == 0:
            tp_pair[0] = ps_big.tile([P, 2 * (VW + 1)], FP16, name="tp", tag="big")
        off = (qt % 2) * (VW + 1)  # 66: keeps the fp16 PSUM slice 4B-aligned
        nc.tensor.transpose(
            tp_pair[0][:, off : off + VW],
            cT_sb[0:VW, qt * P : (qt + 1) * P],
            ident[0:VW, 0:VW],
        )
        r = small_pool.tile([P, 1], F32, name="recip", tag="recip")
        nc.vector.reciprocal(r, tp_pair[0][:, off + DH : off + DH + 1])
        nc.vector.tensor_scalar_mul(
            staging[qt][:, (h2 % HG) * DH : (h2 % HG + 1) * DH],
            tp_pair[0][:, off : off + DH],
            r,
        )
        if h2 % HG == HG - 1:
            nc.sync.dma_start(
                out=o_d.ap()[qt * P : (qt + 1) * P, g * HG * DH : (g + 1) * HG * DH],
                in_=staging[qt],
            )

    # per-head step loop.  At head h: scores/exp(h), ctx^T matmuls (h-1),
    # transpose+normalize+DMA (h-2).
    qkT = emit_proj(0)
    pT_prev, cT_sb_prev = None, None
    for h in range(H):
        jt = h // 2
        if h % 2 == 0 and h > 0:
            qkT = emit_proj(jt)
        qTj, kTj = qkT
        ctx_chunks = list(ctx_mm_pairs(h - 1, pT_prev)) if pT_prev else []
        # finish ctx^T by step 5 so its PSUM->SBUF copy (step 6) frees the
        # ring-1 accumulator before the next head's first ctx^T matmul
        chunk_quota = [2, 2, 1, 1, 1, 1, 0, 0]
        pT = []
        cT_sb_new = None
        for kt in range(NT):
            for _ in range(chunk_quota[kt]):
                if ctx_chunks:
                    ctx_chunks.pop(0)()
            if kt == 6 and pT_prev:
                cT_sb_new = emit_ctxT_copy()
            if cT_sb_prev is not None:
                emit_out_qt(h - 2, cT_sb_prev, kt)
            pT.append(emit_scores_exp_step(h, kt, qTj, kTj))
        assert not ctx_chunks
        cT_sb_prev = cT_sb_new
        pT_prev = pT

    # drain: ctx^T(14) out, ctx^T(15) mm+copy+out
    for kt in range(NT):
        emit_out_qt(H - 2, cT_sb_prev, kt)
    for go in ctx_mm_pairs(H - 1, pT_prev):
        go()
    cT_sb_prev = emit_ctxT_copy()
    for kt in range(NT):
        emit_out_qt(H - 1, cT_sb_prev, kt)


def build_program(n_reps: int = 1, n_loop: int = 0) -> bass.Bass:
    nc = bacc.Bacc(trn_type="TRN2", target_bir_lowering=False, debug=False)

    x_d = nc.declare_dram_parameter("hidden_states", [S, D], F32, isOutput=False)
    m_d = nc.declare_dram_parameter("attention_mask", [S], F32, isOutput=False)
    wq_d = nc.declare_dram_parameter("Wq", [D, D], F32, isOutput=False)
    bq_d = nc.declare_dram_parameter("bq", [D], F32, isOutput=False)
    wk_d = nc.declare_dram_parameter("Wk", [D, D], F32, isOutput=False)
    bk_d = nc.declare_dram_parameter("bk", [D], F32, isOutput=False)
    wv_d = nc.declare_dram_parameter("Wv", [D, D], F32, isOutput=False)
    bv_d = nc.declare_dram_parameter("bv", [D], F32, isOutput=False)
    o_d = nc.declare_dram_parameter("out", [S, D], F32, isOutput=True)
    dram = (x_d, m_d, wq_d, bq_d, wk_d, bk_d, wv_d, bv_d, o_d)

    with tile.TileContext(nc) as tc:
        with (
            tc.tile_pool(name="consts", bufs=1) as cst,
            tc.tile_pool(name="xT", bufs=1) as xT_pool,
            tc.tile_pool(name="qkT", bufs=2) as qkT_pool,
            tc.tile_pool(name="vsb", bufs=1) as v_pool,
            tc.tile_pool(name="wf", bufs=4) as wf_pool,
            tc.tile_pool(name="wb", bufs=1) as wb_pool,
            tc.tile_pool(name="pT", bufs=16) as p_pool,
            tc.tile_pool(name="small", bufs=16) as small_pool,
            tc.tile_pool(name="og", bufs=2) as og_pool,
            tc.tile_pool(name="cT", bufs=2) as cT_pool,
            # PSUM: mixed ring-3 of 2-bank slots + ring-1 ctx^T accumulator
            tc.tile_pool(name="psbig", bufs=3, space="PSUM") as ps_big,
            tc.tile_pool(name="pscT", bufs=1, space="PSUM") as ps_cT,
        ):
            ident = cst.tile([P, P], FP16, name="ident", tag="ident")
            make_identity(nc, ident)
            pools = (cst, xT_pool, qkT_pool, v_pool, wf_pool, wb_pool, p_pool,
                     small_pool, og_pool, cT_pool, ps_big, ps_cT, ident)
            if n_loop:
                with tc.For_i(0, n_loop, 1):
                    emit_body(nc, dram, pools)
            else:
                for _ in range(n_reps):
                    emit_body(nc, dram, pools)
    nc.compile()
    return nc


_NC_CACHE = None


def _get_nc():
    global _NC_CACHE
    if _NC_CACHE is None:
        _NC_CACHE = build_program()
    return _NC_CACHE


def make_in_maps(hidden_states, attention_mask, Wq, bq, Wk, bk, Wv, bv):
    hs = np.ascontiguousarray(np.asarray(hidden_states, dtype=np.float32))
    am = np.ascontiguousarray(
        np.asarray(attention_mask, dtype=np.float32).reshape(B, S)
    )
    shared = {
        "Wq": np.ascontiguousarray(np.asarray(Wq, dtype=np.float32)),
        "bq": np.ascontiguousarray(np.asarray(bq, dtype=np.float32)),
        "Wk": np.ascontiguousarray(np.asarray(Wk, dtype=np.float32)),
        "bk": np.ascontiguousarray(np.asarray(bk, dtype=np.float32)),
        "Wv": np.ascontiguousarray(np.asarray(Wv, dtype=np.float32)),
        "bv": np.ascontiguousarray(np.asarray(bv, dtype=np.float32)),
    }
    return [
        {"hidden_states": hs[b], "attention_mask": am[b], **shared}
        for b in range(B)
    ]


def kernel(hidden_states, attention_mask, Wq, bq, Wk, bk, Wv, bv):
    nc = _get_nc()
    in_maps = make_in_maps(hidden_states, attention_mask, Wq, bq, Wk, bk, Wv, bv)
    res = run_bass_kernel_spmd(nc, in_maps, list(range(N_CORES))).results
    out = np.stack([np.asarray(res[b]["out"], dtype=np.float32) for b in range(B)])
    return out
